# revision 1
# baseline (speedup 1.0000x reference)
import sys
from contextlib import ExitStack

sys.path.insert(0, "/opt/trn_rl_repo")

import numpy as np
import ml_dtypes

import concourse.bass as bass
import concourse.bacc as bacc
import concourse.mybir as mybir
import concourse.tile as tile
from concourse.bass_utils import run_bass_kernel_spmd
from concourse.masks import make_identity

B, N, D, H, HD = 4, 4096, 1024, 16, 64
NCORES = 8
T = (B * N) // NCORES  # 2048 tokens per core
P = 128
NT = T // P            # 16 token tiles per core
KT = D // P            # 8 contraction tiles
E3 = 3 * D

_CACHE = {}


def _name(t):
    return t.name if hasattr(t, "name") else t.tensor.name


def _build():
    bf = mybir.dt.bfloat16
    f32 = mybir.dt.float32
    X = mybir.AxisListType.X
    nc = bacc.Bacc(None, target_bir_lowering=False)
    names = {}
    with tile.TileContext(nc) as tc:
        with ExitStack() as ctx:
            dram = ctx.enter_context(tc.tile_pool(name="dram", bufs=1, space="DRAM"))
            xT_d = dram.tile([D, T], bf, kind="ExternalInput")
            wq_d = dram.tile([D, E3], bf, kind="ExternalInput")
            wo_d = dram.tile([D, D], bf, kind="ExternalInput")
            out_d = dram.tile([T, D], f32, kind="ExternalOutput")
            names["xT"] = _name(xT_d)
            names["wqkvT"] = _name(wq_d)
            names["woT"] = _name(wo_d)
            names["out"] = _name(out_d)

            consts = ctx.enter_context(tc.tile_pool(name="consts", bufs=1))
            xT_sb = consts.tile([P, KT, T], bf)
            wq_sb = consts.tile([P, KT, E3], bf)
            wo_sb = consts.tile([P, KT, D], bf)
            ident = consts.tile([P, P], bf)
            make_identity(nc, ident)
            nc.sync.dma_start(out=xT_sb[:], in_=xT_d[:].rearrange("(k p) t -> p k t", p=P))
            nc.sync.dma_start(out=wq_sb[:], in_=wq_d[:].rearrange("(k p) e -> p k e", p=P))
            nc.sync.dma_start(out=wo_sb[:], in_=wo_d[:].rearrange("(k p) e -> p k e", p=P))

            pool = ctx.enter_context(tc.tile_pool(name="work", bufs=2))
            psum1 = ctx.enter_context(tc.tile_pool(name="psum1", bufs=2, space="PSUM"))
            psum2 = ctx.enter_context(tc.tile_pool(name="psum2", bufs=2, space="PSUM"))
            psum3 = ctx.enter_context(tc.tile_pool(name="psum3", bufs=2, space="PSUM"))

            for i in range(NT):
                tsl = bass.ts(i, P)
                # ---- QKV projection: qkv[t, e] for this 128-token tile ----
                qkv = pool.tile([P, E3], bf, tag="qkv")
                for nch in range(E3 // 512):
                    ps = psum1.tile([P, 512], f32, tag="mm1")
                    for k in range(KT):
                        nc.tensor.matmul(
                            ps[:],
                            xT_sb[:, k, tsl],
                            wq_sb[:, k, bass.ts(nch, 512)],
                            start=(k == 0),
                            stop=(k == KT - 1),
                        )
                    nc.scalar.copy(qkv[:, bass.ts(nch, 512)], ps[:])

                # ---- scores[t, h, g] = sum_d q[t,h,d] k[t,g,d] ----
                qv = qkv[:, 0:D].rearrange("p (h d) -> p h d", d=HD)
                scores = pool.tile([P, H, H], f32, tag="sc")
                tmp = pool.tile([P, H, HD], bf, tag="tmp")
                for g in range(H):
                    kg = qkv[:, D + g * HD : D + (g + 1) * HD]
                    kgb = kg[:, None, :].broadcast_to((P, H, HD))
                    nc.vector.tensor_mul(tmp[:], qv, kgb)
                    nc.vector.reduce_sum(scores[:, :, g : g + 1], tmp[:], axis=X)

                # ---- softmax over g (no max-subtract; |s/32| is small) ----
                we = pool.tile([P, H, H], bf, tag="we")
                den = pool.tile([P, H], f32, tag="den")
                rec = pool.tile([P, H], f32, tag="rec")
                nc.scalar.activation(
                    we[:], scores[:], mybir.ActivationFunctionType.Exp, scale=1.0 / 32.0
                )
                nc.vector.reduce_sum(den[:], we[:], axis=X)
                nc.vector.reciprocal(rec[:], den[:])

                # ---- attn[t, h, d] = sum_g w[t,h,g] v[t,g,d] (unnormalized) ----
                attn = pool.tile([P, H, HD], f32, tag="attn")
                for g in range(H):
                    vg = qkv[:, 2 * D + g * HD : 2 * D + (g + 1) * HD]
                    vgb = vg[:, None, :].broadcast_to((P, H, HD))
                    wgb = we[:, :, g : g + 1].broadcast_to((P, H, HD))
                    if g == 0:
                        nc.vector.tensor_mul(attn[:], wgb, vgb)
                    else:
                        nc.vector.tensor_mul(tmp[:], wgb, vgb)
                        nc.vector.tensor_add(attn[:], attn[:], tmp[:])

                # ---- normalize rows by 1/den per (t, h), cast to bf16 ----
                attnb = pool.tile([P, H, HD], bf, tag="attnb")
                for h in range(H):
                    nc.vector.tensor_scalar_mul(
                        attnb[:, h, :], attn[:, h, :], rec[:, h : h + 1]
                    )

                # ---- transpose attn tile -> [e, t] blocks for output proj ----
                attnb_flat = attnb.rearrange("p h d -> p (h d)")
                attnT = pool.tile([P, KT, P], bf, tag="attnT")
                for c in range(KT):
                    pt = psum2.tile([P, P], bf, tag="pt")
                    nc.tensor.transpose(pt[:], attnb_flat[:, bass.ts(c, P)], ident[:])
                    nc.scalar.copy(attnT[:, c, :], pt[:])

                # ---- output projection ----
                outt = pool.tile([P, D], f32, tag="outt")
                for nch in range(D // 512):
                    po = psum3.tile([P, 512], f32, tag="po")
                    for k in range(KT):
                        nc.tensor.matmul(
                            po[:],
                            attnT[:, k, :],
                            wo_sb[:, k, bass.ts(nch, 512)],
                            start=(k == 0),
                            stop=(k == KT - 1),
                        )
                    nc.scalar.copy(outt[:, bass.ts(nch, 512)], po[:])
                nc.sync.dma_start(out=out_d[tsl, :], in_=outt[:])
    nc.compile()
    return nc, names


def kernel(x, Wqkv, Wo, bo, trace=False):
    if "nc" not in _CACHE:
        _CACHE["nc"], _CACHE["names"] = _build()
    nc, names = _CACHE["nc"], _CACHE["names"]
    bf = ml_dtypes.bfloat16
    xt = np.ascontiguousarray(
        np.asarray(x, dtype=np.float32).reshape(B * N, D).T
    )  # [D, B*N]
    wqkvT = np.ascontiguousarray(np.asarray(Wqkv, dtype=np.float32).T).astype(bf)
    woT = np.ascontiguousarray(np.asarray(Wo, dtype=np.float32).T).astype(bf)
    in_maps = []
    for c in range(NCORES):
        shard = np.ascontiguousarray(xt[:, c * T : (c + 1) * T]).astype(bf)
        in_maps.append(
            {names["xT"]: shard, names["wqkvT"]: wqkvT, names["woT"]: woT}
        )
    res = run_bass_kernel_spmd(
        nc, in_maps, core_ids=list(range(NCORES)), trace=trace
    )
    shards = [res.results[c][names["out"]] for c in range(NCORES)]
    out = np.concatenate(shards, axis=0).reshape(B, N, D).astype(np.float32)
    out = out + np.asarray(bo, dtype=np.float32)[None, None, :]
    if trace:
        return out, res
    return out



# revision 2
# speedup vs baseline: 1.0135x; 1.0135x over previous
import sys
from contextlib import ExitStack

sys.path.insert(0, "/opt/trn_rl_repo")

import numpy as np
import ml_dtypes

import concourse.bass as bass
import concourse.bacc as bacc
import concourse.mybir as mybir
import concourse.tile as tile
from concourse.bass_utils import run_bass_kernel_spmd
from concourse.masks import make_identity

B, N, D, H = 4, 4096, 1024, 16
HD = D // H
NCORES = 8
T = (B * N) // NCORES  # 2048 tokens per core
P = 128
NT = T // P            # 16 token tiles per core
KT = D // P            # 8 contraction tiles
E3 = 3 * D

GP_ATTN = 10  # attn g-groups offloaded to gpsimd (of H=16)

_CACHE = {}


def _name(t):
    return t.name if hasattr(t, "name") else t.tensor.name


def _build():
    bf = mybir.dt.bfloat16
    f32 = mybir.dt.float32
    X = mybir.AxisListType.X
    nc = bacc.Bacc(None, target_bir_lowering=False)
    names = {}
    with tile.TileContext(nc) as tc:
        with ExitStack() as ctx:
            dram = ctx.enter_context(tc.tile_pool(name="dram", bufs=1, space="DRAM"))
            xT_d = dram.tile([D, T], bf, kind="ExternalInput")
            wq_d = dram.tile([D, E3], bf, kind="ExternalInput")
            wo_d = dram.tile([D, D], bf, kind="ExternalInput")
            out_d = dram.tile([T, D], f32, kind="ExternalOutput")
            names["xT"] = _name(xT_d)
            names["wqkvT"] = _name(wq_d)
            names["woT"] = _name(wo_d)
            names["out"] = _name(out_d)

            consts = ctx.enter_context(tc.tile_pool(name="consts", bufs=1))
            xT_sb = consts.tile([P, KT, T], bf)
            wq_sb = consts.tile([P, KT, E3], bf)
            wo_sb = consts.tile([P, KT, D], bf)
            ident = consts.tile([P, P], bf)
            make_identity(nc, ident)
            nc.sync.dma_start(out=xT_sb[:], in_=xT_d[:].rearrange("(k p) t -> p k t", p=P))
            nc.sync.dma_start(out=wq_sb[:], in_=wq_d[:].rearrange("(k p) e -> p k e", p=P))
            nc.sync.dma_start(out=wo_sb[:], in_=wo_d[:].rearrange("(k p) e -> p k e", p=P))

            pool = ctx.enter_context(tc.tile_pool(name="work", bufs=2))
            psum1 = ctx.enter_context(tc.tile_pool(name="psum1", bufs=2, space="PSUM"))
            psum2 = ctx.enter_context(tc.tile_pool(name="psum2", bufs=2, space="PSUM"))
            psum3 = ctx.enter_context(tc.tile_pool(name="psum3", bufs=2, space="PSUM"))

            # stage1(i): qkv proj + score muls + reduces + softmax -> wn
            # stage2(i): attn (DVE+GP) + merge + transpose + out proj
            # Emitted as s1(0), s1(1), s2(0), s1(2), s2(1), ... so DVE always
            # has stage-1 work queued while GPSIMD chews on stage-2 groups.
            stage2_state = {}

            def stage1(i):
                tsl = bass.ts(i, P)
                qkv = pool.tile([P, E3], bf, tag="qkv")
                for nch in range(E3 // 512):
                    ps = psum1.tile([P, 512], f32, tag="mm1")
                    for k in range(KT):
                        nc.tensor.matmul(
                            ps[:],
                            xT_sb[:, k, tsl],
                            wq_sb[:, k, bass.ts(nch, 512)],
                            start=(k == 0),
                            stop=(k == KT - 1),
                        )
                    nc.scalar.copy(qkv[:, bass.ts(nch, 512)], ps[:])

                qv = qkv[:, 0:D].rearrange("p (h d) -> p h d", d=HD)
                # products for all g, then two batched reduces
                tmpbig = pool.tile([P, H, H, HD], bf, tag="tmpbig")
                for g in range(H):
                    kg = qkv[:, D + g * HD : D + (g + 1) * HD]
                    kgb = kg[:, None, :].broadcast_to((P, H, HD))
                    nc.vector.tensor_mul(tmpbig[:, g], qv, kgb)
                scores = pool.tile([P, H, H], f32, tag="sc")  # [p, g, h]
                nc.vector.reduce_sum(scores[:, 0:8, :, None], tmpbig[:, 0:8], axis=X)
                nc.vector.reduce_sum(scores[:, 8:16, :, None], tmpbig[:, 8:16], axis=X)

                # softmax over g (outer axis of [p, g, h]); no max-subtract
                we = pool.tile([P, H, H], bf, tag="we")  # exp scores [p, g, h]
                den = pool.tile([P, H], f32, tag="den")
                rec = pool.tile([P, H], f32, tag="rec")
                wn = pool.tile([P, H, H], bf, tag="wn")  # normalized w [p, g, h]
                nc.scalar.activation(
                    we[:], scores[:], mybir.ActivationFunctionType.Exp, scale=1.0 / 32.0
                )
                # den[p, h] = sum_g we[p, g, h] : strided view, innermost = g
                nc.vector.reduce_sum(
                    den[:, :, None], we[:].rearrange("p g h -> p h g"), axis=X
                )
                nc.vector.reciprocal(rec[:], den[:])
                recb = rec[:, None, :].broadcast_to((P, H, H))
                nc.vector.tensor_mul(wn[:], we[:], recb)
                stage2_state[i] = (qkv, wn)

            def stage2(i):
                tsl = bass.ts(i, P)
                qkv, wn = stage2_state.pop(i)
                # attn[p, h, d] = sum_g wn[p, g, h] * v[p, g, d]
                n_dve = H - GP_ATTN
                attn_dve = pool.tile([P, H, HD], bf, tag="attn_dve")
                tmp = pool.tile([P, H, HD], bf, tag="tmp")
                attn_gp = pool.tile([P, H, HD], bf, tag="attn_gp")
                tmp_gp = pool.tile([P, H, HD], bf, tag="tmp_gp")

                def emit_group(eng, g, acc, tmp_t, first):
                    vg = qkv[:, 2 * D + g * HD : 2 * D + (g + 1) * HD]
                    vgb = vg[:, None, :].broadcast_to((P, H, HD))
                    wgb = wn[:, g, :, None].broadcast_to((P, H, HD))
                    if first:
                        eng.tensor_mul(acc[:], wgb, vgb)
                    else:
                        eng.tensor_mul(tmp_t[:], wgb, vgb)
                        eng.tensor_add(acc[:], acc[:], tmp_t[:])

                for j, g in enumerate(range(n_dve)):
                    emit_group(nc.vector, g, attn_dve, tmp, j == 0)
                for j, g in enumerate(range(n_dve, H)):
                    emit_group(nc.gpsimd, g, attn_gp, tmp_gp, j == 0)
                if GP_ATTN > 0:
                    nc.vector.tensor_add(attn_dve[:], attn_dve[:], attn_gp[:])

                # transpose attn tile -> [e, t] blocks for output proj
                attnb_flat = attn_dve.rearrange("p h d -> p (h d)")
                attnT = pool.tile([P, KT, P], bf, tag="attnT")
                for c in range(KT):
                    pt = psum2.tile([P, P], bf, tag="pt")
                    nc.tensor.transpose(pt[:], attnb_flat[:, bass.ts(c, P)], ident[:])
                    nc.scalar.copy(attnT[:, c, :], pt[:])

                outt = pool.tile([P, D], f32, tag="outt")
                for nch in range(D // 512):
                    po = psum3.tile([P, 512], f32, tag="po")
                    for k in range(KT):
                        nc.tensor.matmul(
                            po[:],
                            attnT[:, k, :],
                            wo_sb[:, k, bass.ts(nch, 512)],
                            start=(k == 0),
                            stop=(k == KT - 1),
                        )
                    nc.scalar.copy(outt[:, bass.ts(nch, 512)], po[:])
                nc.sync.dma_start(out=out_d[tsl, :], in_=outt[:])

            stage1(0)
            for i in range(1, NT):
                stage1(i)
                stage2(i - 1)
            stage2(NT - 1)
    nc.compile()
    return nc, names


def kernel(x, Wqkv, Wo, bo, trace=False):
    if "nc" not in _CACHE:
        _CACHE["nc"], _CACHE["names"] = _build()
    nc, names = _CACHE["nc"], _CACHE["names"]
    bf = ml_dtypes.bfloat16
    xt = np.ascontiguousarray(
        np.asarray(x, dtype=np.float32).reshape(B * N, D).T
    )  # [D, B*N]
    wqkvT = np.ascontiguousarray(np.asarray(Wqkv, dtype=np.float32).T).astype(bf)
    woT = np.ascontiguousarray(np.asarray(Wo, dtype=np.float32).T).astype(bf)
    in_maps = []
    for c in range(NCORES):
        shard = np.ascontiguousarray(xt[:, c * T : (c + 1) * T]).astype(bf)
        in_maps.append(
            {names["xT"]: shard, names["wqkvT"]: wqkvT, names["woT"]: woT}
        )
    res = run_bass_kernel_spmd(
        nc, in_maps, core_ids=list(range(NCORES)), trace=trace
    )
    shards = [res.results[c][names["out"]] for c in range(NCORES)]
    out = np.concatenate(shards, axis=0).reshape(B, N, D).astype(np.float32)
    out = out + np.asarray(bo, dtype=np.float32)[None, None, :]
    if trace:
        return out, res
    return out


# revision 3
# speedup vs baseline: 1.3760x; 1.3577x over previous
import sys
from contextlib import ExitStack

sys.path.insert(0, "/opt/trn_rl_repo")

import numpy as np
import ml_dtypes

import concourse.bass as bass
import concourse.bacc as bacc
import concourse.mybir as mybir
import concourse.tile as tile
from concourse.bass_utils import run_bass_kernel_spmd
from concourse.masks import make_identity

B, N, D, H = 4, 4096, 1024, 16
HD = D // H
NCORES = 8
T = (B * N) // NCORES  # 2048 tokens per core
P = 128
NT = T // P            # 16 token tiles per core
KT = D // P            # 8 contraction tiles
E3 = 3 * D

GP_ATTN = 0   # gpsimd offload disabled: SBUF-BW contention nets zero

_CACHE = {}


def _name(t):
    return t.name if hasattr(t, "name") else t.tensor.name


def _build():
    bf = mybir.dt.bfloat16
    f32 = mybir.dt.float32
    X = mybir.AxisListType.X
    nc = bacc.Bacc(None, target_bir_lowering=False)
    names = {}
    with tile.TileContext(nc) as tc:
        with ExitStack() as ctx:
            dram = ctx.enter_context(tc.tile_pool(name="dram", bufs=1, space="DRAM"))
            xT_d = dram.tile([D, T], bf, kind="ExternalInput")
            wq_d = dram.tile([D, E3], bf, kind="ExternalInput")
            wo_d = dram.tile([D, D], bf, kind="ExternalInput")
            out_d = dram.tile([T, D], f32, kind="ExternalOutput")
            names["xT"] = _name(xT_d)
            names["wqkvT"] = _name(wq_d)
            names["woT"] = _name(wo_d)
            names["out"] = _name(out_d)

            consts = ctx.enter_context(tc.tile_pool(name="consts", bufs=1))
            xT_sb = consts.tile([P, KT, T], bf)
            wq_sb = consts.tile([P, KT, E3], bf)
            wo_sb = consts.tile([P, KT, D], bf)
            ident = consts.tile([P, P], bf)
            make_identity(nc, ident)
            nc.sync.dma_start(out=xT_sb[:], in_=xT_d[:].rearrange("(k p) t -> p k t", p=P))
            nc.sync.dma_start(out=wq_sb[:], in_=wq_d[:].rearrange("(k p) e -> p k e", p=P))
            nc.sync.dma_start(out=wo_sb[:], in_=wo_d[:].rearrange("(k p) e -> p k e", p=P))

            pool = ctx.enter_context(tc.tile_pool(name="work", bufs=2))
            scratch = ctx.enter_context(tc.tile_pool(name="scratch", bufs=1))
            psum1 = ctx.enter_context(tc.tile_pool(name="psum1", bufs=2, space="PSUM"))
            psum2 = ctx.enter_context(tc.tile_pool(name="psum2", bufs=2, space="PSUM"))
            psum3 = ctx.enter_context(tc.tile_pool(name="psum3", bufs=2, space="PSUM"))

            # stage1(i): qkv proj + score muls + reduces + softmax -> wn
            # stage2(i): attn (DVE+GP) + merge + transpose + out proj
            # Emitted as s1(0), s1(1), s2(0), s1(2), s2(1), ... so DVE always
            # has stage-1 work queued while GPSIMD chews on stage-2 groups.
            stage2_state = {}

            def stage1(i):
                tsl = bass.ts(i, P)
                qkv = pool.tile([P, E3], bf, tag="qkv")
                for nch in range(E3 // 512):
                    ps = psum1.tile([P, 512], f32, tag="mm1")
                    for k in range(KT):
                        nc.tensor.matmul(
                            ps[:],
                            xT_sb[:, k, tsl],
                            wq_sb[:, k, bass.ts(nch, 512)],
                            start=(k == 0),
                            stop=(k == KT - 1),
                        )
                    nc.scalar.copy(qkv[:, bass.ts(nch, 512)], ps[:])

                qv = qkv[:, 0:D].rearrange("p (h d) -> p h d", d=HD)
                # products for all g, then 2x-mode pairwise-add tree over d
                tmpbig = scratch.tile([P, H, H, HD], bf, tag="tmpbig")
                for g in range(H):
                    kg = qkv[:, D + g * HD : D + (g + 1) * HD]
                    kgb = kg[:, None, :].broadcast_to((P, H, HD))
                    nc.vector.tensor_mul(tmpbig[:, g], qv, kgb)
                t1 = scratch.tile([P, H, H, 32], bf, tag="t1")
                t2 = scratch.tile([P, H, H, 16], bf, tag="t2")
                t3 = scratch.tile([P, H, H, 8], bf, tag="t3")
                t4 = scratch.tile([P, H, H, 4], bf, tag="t4")
                t5 = scratch.tile([P, H, H, 2], bf, tag="t5")
                nc.vector.tensor_add(t1[:], tmpbig[:, :, :, 0:32], tmpbig[:, :, :, 32:64])
                nc.vector.tensor_add(t2[:], t1[:, :, :, 0:16], t1[:, :, :, 16:32])
                nc.vector.tensor_add(t3[:], t2[:, :, :, 0:8], t2[:, :, :, 8:16])
                nc.vector.tensor_add(t4[:], t3[:, :, :, 0:4], t3[:, :, :, 4:8])
                nc.vector.tensor_add(t5[:], t4[:, :, :, 0:2], t4[:, :, :, 2:4])
                scores = pool.tile([P, H, H], f32, tag="sc")  # [p, g, h]
                nc.vector.tensor_add(
                    scores[:, :, :, None], t5[:, :, :, 0:1], t5[:, :, :, 1:2]
                )

                # softmax over g (outer axis of [p, g, h]); no max-subtract
                we = pool.tile([P, H, H], bf, tag="we")  # exp scores [p, g, h]
                den = pool.tile([P, H], f32, tag="den")
                rec = pool.tile([P, H], f32, tag="rec")
                wn = pool.tile([P, H, H], bf, tag="wn")  # normalized w [p, g, h]
                nc.scalar.activation(
                    we[:], scores[:], mybir.ActivationFunctionType.Exp, scale=1.0 / 32.0
                )
                # den[p, h] = sum_g we[p, g, h] : strided view, innermost = g
                nc.vector.reduce_sum(
                    den[:, :, None], we[:].rearrange("p g h -> p h g"), axis=X
                )
                nc.vector.reciprocal(rec[:], den[:])
                recb = rec[:, None, :].broadcast_to((P, H, H))
                nc.vector.tensor_mul(wn[:], we[:], recb)
                stage2_state[i] = (qkv, wn)

            def stage2(i):
                tsl = bass.ts(i, P)
                qkv, wn = stage2_state.pop(i)
                # attn[p, h, d] = sum_g wn[p, g, h] * v[p, g, d]
                n_dve = H - GP_ATTN
                attn_dve = pool.tile([P, H, HD], bf, tag="attn_dve")
                tmp = pool.tile([P, H, HD], bf, tag="tmp")
                attn_gp = pool.tile([P, H, HD], bf, tag="attn_gp")
                tmp_gp = pool.tile([P, H, HD], bf, tag="tmp_gp")

                def emit_group(eng, g, acc, tmp_t, first):
                    vg = qkv[:, 2 * D + g * HD : 2 * D + (g + 1) * HD]
                    vgb = vg[:, None, :].broadcast_to((P, H, HD))
                    wgb = wn[:, g, :, None].broadcast_to((P, H, HD))
                    if first:
                        eng.tensor_mul(acc[:], wgb, vgb)
                    else:
                        eng.tensor_mul(tmp_t[:], wgb, vgb)
                        eng.tensor_add(acc[:], acc[:], tmp_t[:])

                for j, g in enumerate(range(n_dve)):
                    emit_group(nc.vector, g, attn_dve, tmp, j == 0)
                for j, g in enumerate(range(n_dve, H)):
                    emit_group(nc.gpsimd, g, attn_gp, tmp_gp, j == 0)
                if GP_ATTN > 0:
                    nc.vector.tensor_add(attn_dve[:], attn_dve[:], attn_gp[:])

                # transpose attn tile -> [e, t] blocks for output proj
                attnb_flat = attn_dve.rearrange("p h d -> p (h d)")
                attnT = pool.tile([P, KT, P], bf, tag="attnT")
                for c in range(KT):
                    pt = psum2.tile([P, P], bf, tag="pt")
                    nc.tensor.transpose(pt[:], attnb_flat[:, bass.ts(c, P)], ident[:])
                    nc.scalar.copy(attnT[:, c, :], pt[:])

                outt = pool.tile([P, D], f32, tag="outt")
                for nch in range(D // 512):
                    po = psum3.tile([P, 512], f32, tag="po")
                    for k in range(KT):
                        nc.tensor.matmul(
                            po[:],
                            attnT[:, k, :],
                            wo_sb[:, k, bass.ts(nch, 512)],
                            start=(k == 0),
                            stop=(k == KT - 1),
                        )
                    nc.scalar.copy(outt[:, bass.ts(nch, 512)], po[:])
                nc.sync.dma_start(out=out_d[tsl, :], in_=outt[:])

            stage1(0)
            for i in range(1, NT):
                stage1(i)
                stage2(i - 1)
            stage2(NT - 1)
    nc.compile()
    return nc, names


def kernel(x, Wqkv, Wo, bo, trace=False):
    if "nc" not in _CACHE:
        _CACHE["nc"], _CACHE["names"] = _build()
    nc, names = _CACHE["nc"], _CACHE["names"]
    bf = ml_dtypes.bfloat16
    xt = np.ascontiguousarray(
        np.asarray(x, dtype=np.float32).reshape(B * N, D).T
    )  # [D, B*N]
    wqkvT = np.ascontiguousarray(np.asarray(Wqkv, dtype=np.float32).T).astype(bf)
    woT = np.ascontiguousarray(np.asarray(Wo, dtype=np.float32).T).astype(bf)
    in_maps = []
    for c in range(NCORES):
        shard = np.ascontiguousarray(xt[:, c * T : (c + 1) * T]).astype(bf)
        in_maps.append(
            {names["xT"]: shard, names["wqkvT"]: wqkvT, names["woT"]: woT}
        )
    res = run_bass_kernel_spmd(
        nc, in_maps, core_ids=list(range(NCORES)), trace=trace
    )
    shards = [res.results[c][names["out"]] for c in range(NCORES)]
    out = np.concatenate(shards, axis=0).reshape(B, N, D).astype(np.float32)
    out = out + np.asarray(bo, dtype=np.float32)[None, None, :]
    if trace:
        return out, res
    return out


# revision 4
# speedup vs baseline: 1.3991x; 1.0168x over previous
import sys
from contextlib import ExitStack

sys.path.insert(0, "/opt/trn_rl_repo")

import numpy as np
import ml_dtypes

import concourse.bass as bass
import concourse.bacc as bacc
import concourse.mybir as mybir
import concourse.tile as tile
from concourse.bass_utils import run_bass_kernel_spmd
from concourse.masks import make_identity

B, N, D, H = 4, 4096, 1024, 16
HD = D // H
NCORES = 8
T = (B * N) // NCORES  # 2048 tokens per core
P = 128
NT = T // P            # 16 token tiles per core
KT = D // P            # 8 contraction tiles
E3 = 3 * D

GP_ATTN = 0   # gpsimd offload disabled: SBUF-BW contention nets zero

_CACHE = {}


def _name(t):
    return t.name if hasattr(t, "name") else t.tensor.name


def _build():
    bf = mybir.dt.bfloat16
    f32 = mybir.dt.float32
    X = mybir.AxisListType.X
    nc = bacc.Bacc(None, target_bir_lowering=False)
    names = {}
    with tile.TileContext(nc) as tc:
        with ExitStack() as ctx:
            dram = ctx.enter_context(tc.tile_pool(name="dram", bufs=1, space="DRAM"))
            xT_d = dram.tile([D, T], bf, kind="ExternalInput")
            wq_d = dram.tile([D, E3], bf, kind="ExternalInput")
            wo_d = dram.tile([D, D], bf, kind="ExternalInput")
            out_d = dram.tile([T, D], f32, kind="ExternalOutput")
            names["xT"] = _name(xT_d)
            names["wqkvT"] = _name(wq_d)
            names["woT"] = _name(wo_d)
            names["out"] = _name(out_d)

            consts = ctx.enter_context(tc.tile_pool(name="consts", bufs=1))
            xT_sb = consts.tile([P, KT, T], bf)
            wq_sb = consts.tile([P, KT, E3], bf)
            wo_sb = consts.tile([P, KT, D], bf)
            ident = consts.tile([P, P], bf)
            make_identity(nc, ident)
            nc.sync.dma_start(out=xT_sb[:], in_=xT_d[:].rearrange("(k p) t -> p k t", p=P))
            nc.sync.dma_start(out=wq_sb[:], in_=wq_d[:].rearrange("(k p) e -> p k e", p=P))
            nc.sync.dma_start(out=wo_sb[:], in_=wo_d[:].rearrange("(k p) e -> p k e", p=P))

            pool = ctx.enter_context(tc.tile_pool(name="work", bufs=2))
            scratch = ctx.enter_context(tc.tile_pool(name="scratch", bufs=1))
            psum1 = ctx.enter_context(tc.tile_pool(name="psum1", bufs=2, space="PSUM"))
            psum2 = ctx.enter_context(tc.tile_pool(name="psum2", bufs=2, space="PSUM"))
            psum3 = ctx.enter_context(tc.tile_pool(name="psum3", bufs=2, space="PSUM"))

            # stage1(i): qkv proj + score muls + reduces + softmax -> wn
            # stage2(i): attn (DVE+GP) + merge + transpose + out proj
            # Emitted as s1(0), s1(1), s2(0), s1(2), s2(1), ... so DVE always
            # has stage-1 work queued while GPSIMD chews on stage-2 groups.
            stage2_state = {}

            def stage1(i):
                tsl = bass.ts(i, P)
                qkv = pool.tile([P, E3], bf, tag="qkv")
                for nch in range(E3 // 512):
                    ps = psum1.tile([P, 512], f32, tag="mm1")
                    for k in range(KT):
                        nc.tensor.matmul(
                            ps[:],
                            xT_sb[:, k, tsl],
                            wq_sb[:, k, bass.ts(nch, 512)],
                            start=(k == 0),
                            stop=(k == KT - 1),
                        )
                    nc.scalar.copy(qkv[:, bass.ts(nch, 512)], ps[:])

                qv3 = qkv[:, 0:D].rearrange("p (h d) -> p h d", d=HD)
                kv3 = qkv[:, D : 2 * D].rearrange("p (g d) -> p g d", d=HD)
                # one 2x-mode mul for all (h,g) products, then 2x pairwise tree
                big = scratch.tile([P, H * H * HD], bf, tag="big")
                t1 = scratch.tile([P, 8192], bf, tag="t1")
                t2 = scratch.tile([P, 4096], bf, tag="t2")
                t3 = scratch.tile([P, 2048], bf, tag="t3")
                t4 = scratch.tile([P, 1024], bf, tag="t4")
                t5 = scratch.tile([P, 512], bf, tag="t5")
                bigS = big[:].rearrange("p (h g d) -> p h g d", h=H, g=H)
                nc.vector.tensor_mul(
                    bigS,
                    qv3[:, :, None, :].broadcast_to((P, H, H, HD)),
                    kv3[:, None, :, :].broadcast_to((P, H, H, HD)),
                )
                T1 = t1[:].rearrange("p (h g d) -> p h g d", h=H, g=H)
                T2 = t2[:].rearrange("p (h g d) -> p h g d", h=H, g=H)
                T3 = t3[:].rearrange("p (h g d) -> p h g d", h=H, g=H)
                T4 = t4[:].rearrange("p (h g d) -> p h g d", h=H, g=H)
                T5 = t5[:].rearrange("p (h g d) -> p h g d", h=H, g=H)
                nc.vector.tensor_add(T1, bigS[:, :, :, 0:32], bigS[:, :, :, 32:64])
                nc.vector.tensor_add(T2, T1[:, :, :, 0:16], T1[:, :, :, 16:32])
                nc.vector.tensor_add(T3, T2[:, :, :, 0:8], T2[:, :, :, 8:16])
                nc.vector.tensor_add(T4, T3[:, :, :, 0:4], T3[:, :, :, 4:8])
                nc.vector.tensor_add(T5, T4[:, :, :, 0:2], T4[:, :, :, 2:4])
                scores = pool.tile([P, H, H], f32, tag="sc")  # [p, h, g]
                nc.vector.tensor_add(
                    scores[:, :, :, None], T5[:, :, :, 0:1], T5[:, :, :, 1:2]
                )

                # softmax over g (innermost of [p, h, g]); no max-subtract
                we = pool.tile([P, H, H], bf, tag="we")  # exp scores [p, h, g]
                den = pool.tile([P, H], f32, tag="den")
                rec = pool.tile([P, H], f32, tag="rec")
                wn = pool.tile([P, H, H], bf, tag="wn")  # normalized w [p, h, g]
                nc.scalar.activation(
                    we[:], scores[:], mybir.ActivationFunctionType.Exp, scale=1.0 / 32.0
                )
                nc.vector.reduce_sum(den[:, :, None], we[:], axis=X)
                nc.vector.reciprocal(rec[:], den[:])
                recb = rec[:, :, None].broadcast_to((P, H, H))
                nc.vector.tensor_mul(wn[:], we[:], recb)
                stage2_state[i] = (qkv, wn, big, t1, t2, t3)

            def stage2(i):
                tsl = bass.ts(i, P)
                qkv, wn, big, t1, t2, t3 = stage2_state.pop(i)
                # attn[p, h, d] = sum_g wn[p, h, g] * v[p, d, g]
                # (host pre-permuted the V weight block so v lands as [d, g])
                vv = qkv[:, 2 * D : 3 * D].rearrange("p (d g) -> p d g", g=H)
                bigA = big[:].rearrange("p (h d g) -> p h d g", h=H, d=HD)
                nc.vector.tensor_mul(
                    bigA,
                    wn[:, :, None, :].broadcast_to((P, H, HD, H)),
                    vv[:, None, :, :].broadcast_to((P, H, HD, H)),
                )
                A1 = t1[:].rearrange("p (h d g) -> p h d g", h=H, d=HD)
                A2 = t2[:].rearrange("p (h d g) -> p h d g", h=H, d=HD)
                A3 = t3[:].rearrange("p (h d g) -> p h d g", h=H, d=HD)
                attn_dve = pool.tile([P, H, HD], bf, tag="attn_dve")
                nc.vector.tensor_add(A1, bigA[:, :, :, 0:8], bigA[:, :, :, 8:16])
                nc.vector.tensor_add(A2, A1[:, :, :, 0:4], A1[:, :, :, 4:8])
                nc.vector.tensor_add(A3, A2[:, :, :, 0:2], A2[:, :, :, 2:4])
                nc.vector.tensor_add(
                    attn_dve[:, :, :, None], A3[:, :, :, 0:1], A3[:, :, :, 1:2]
                )

                # transpose attn tile -> [e, t] blocks for output proj
                attnb_flat = attn_dve.rearrange("p h d -> p (h d)")
                attnT = pool.tile([P, KT, P], bf, tag="attnT")
                for c in range(KT):
                    pt = psum2.tile([P, P], bf, tag="pt")
                    nc.tensor.transpose(pt[:], attnb_flat[:, bass.ts(c, P)], ident[:])
                    nc.scalar.copy(attnT[:, c, :], pt[:])

                outt = pool.tile([P, D], f32, tag="outt")
                for nch in range(D // 512):
                    po = psum3.tile([P, 512], f32, tag="po")
                    for k in range(KT):
                        nc.tensor.matmul(
                            po[:],
                            attnT[:, k, :],
                            wo_sb[:, k, bass.ts(nch, 512)],
                            start=(k == 0),
                            stop=(k == KT - 1),
                        )
                    nc.scalar.copy(outt[:, bass.ts(nch, 512)], po[:])
                nc.sync.dma_start(out=out_d[tsl, :], in_=outt[:])

            stage1(0)
            for i in range(1, NT):
                stage1(i)
                stage2(i - 1)
            stage2(NT - 1)
    nc.compile()
    return nc, names


def kernel(x, Wqkv, Wo, bo, trace=False):
    if "nc" not in _CACHE:
        _CACHE["nc"], _CACHE["names"] = _build()
    nc, names = _CACHE["nc"], _CACHE["names"]
    bf = ml_dtypes.bfloat16
    xt = np.ascontiguousarray(
        np.asarray(x, dtype=np.float32).reshape(B * N, D).T
    )  # [D, B*N]
    wqkvT_f = np.ascontiguousarray(np.asarray(Wqkv, dtype=np.float32).T)  # [D, 3D]
    vblk = wqkvT_f[:, 2 * D :].reshape(D, H, HD)
    wqkvT_f[:, 2 * D :] = np.ascontiguousarray(vblk.transpose(0, 2, 1)).reshape(D, D)
    wqkvT = wqkvT_f.astype(bf)
    woT = np.ascontiguousarray(np.asarray(Wo, dtype=np.float32).T).astype(bf)
    in_maps = []
    for c in range(NCORES):
        shard = np.ascontiguousarray(xt[:, c * T : (c + 1) * T]).astype(bf)
        in_maps.append(
            {names["xT"]: shard, names["wqkvT"]: wqkvT, names["woT"]: woT}
        )
    res = run_bass_kernel_spmd(
        nc, in_maps, core_ids=list(range(NCORES)), trace=trace
    )
    shards = [res.results[c][names["out"]] for c in range(NCORES)]
    out = np.concatenate(shards, axis=0).reshape(B, N, D).astype(np.float32)
    out = out + np.asarray(bo, dtype=np.float32)[None, None, :]
    if trace:
        return out, res
    return out


# revision 5
# speedup vs baseline: 1.6836x; 1.2033x over previous
import sys
from contextlib import ExitStack

sys.path.insert(0, "/opt/trn_rl_repo")

import numpy as np
import ml_dtypes

import concourse.bass as bass
import concourse.bacc as bacc
import concourse.mybir as mybir
import concourse.tile as tile
from concourse.bass_utils import run_bass_kernel_spmd
from concourse.masks import make_identity

B, N, D, H = 4, 4096, 1024, 16
HD = D // H
NCORES = 8
T = (B * N) // NCORES  # 2048 tokens per core
P = 128
NT = T // P            # 16 token tiles per core
KT = D // P            # 8 contraction tiles
E3 = 3 * D

GP_ATTN = 0   # gpsimd offload disabled: SBUF-BW contention nets zero

_CACHE = {}


def _name(t):
    return t.name if hasattr(t, "name") else t.tensor.name


def _build():
    bf = mybir.dt.bfloat16
    f32 = mybir.dt.float32
    X = mybir.AxisListType.X
    nc = bacc.Bacc(None, target_bir_lowering=False)
    names = {}
    with tile.TileContext(nc) as tc:
        with ExitStack() as ctx:
            dram = ctx.enter_context(tc.tile_pool(name="dram", bufs=1, space="DRAM"))
            xT_d = dram.tile([D, T], bf, kind="ExternalInput")
            wq_d = dram.tile([D, E3], bf, kind="ExternalInput")
            wo_d = dram.tile([D, D], bf, kind="ExternalInput")
            out_d = dram.tile([T, D], f32, kind="ExternalOutput")
            names["xT"] = _name(xT_d)
            names["wqkvT"] = _name(wq_d)
            names["woT"] = _name(wo_d)
            names["out"] = _name(out_d)

            consts = ctx.enter_context(tc.tile_pool(name="consts", bufs=1))
            xT_sb = consts.tile([P, KT, T], bf)
            wq_sb = consts.tile([P, KT, E3], bf)
            wo_sb = consts.tile([P, KT, D], bf)
            ident = consts.tile([P, P], bf)
            make_identity(nc, ident)
            # split input DMAs so tile-0 compute starts as soon as its
            # slices land (whole-tensor uploads cost ~47us of head idle)
            for nch in range(E3 // 512):
                esl = bass.ts(nch, 512)
                nc.sync.dma_start(
                    out=wq_sb[:, :, esl],
                    in_=wq_d[:, esl].rearrange("(k p) e -> p k e", p=P),
                )
            for i in range(NT):
                tsl = bass.ts(i, P)
                nc.sync.dma_start(
                    out=xT_sb[:, :, tsl],
                    in_=xT_d[:, tsl].rearrange("(k p) t -> p k t", p=P),
                )
            nc.sync.dma_start(out=wo_sb[:], in_=wo_d[:].rearrange("(k p) e -> p k e", p=P))

            pool = ctx.enter_context(tc.tile_pool(name="work", bufs=2))
            scratch = ctx.enter_context(tc.tile_pool(name="scratch", bufs=1))
            psum1 = ctx.enter_context(tc.tile_pool(name="psum1", bufs=2, space="PSUM"))
            psum2 = ctx.enter_context(tc.tile_pool(name="psum2", bufs=2, space="PSUM"))
            psum3 = ctx.enter_context(tc.tile_pool(name="psum3", bufs=2, space="PSUM"))

            # stage1(i): qkv proj + score muls + reduces + softmax -> wn
            # stage2(i): attn (DVE+GP) + merge + transpose + out proj
            # Emitted as s1(0), s1(1), s2(0), s1(2), s2(1), ... so DVE always
            # has stage-1 work queued while GPSIMD chews on stage-2 groups.
            stage2_state = {}

            def stage1(i):
                tsl = bass.ts(i, P)
                qkv = pool.tile([P, E3], bf, tag="qkv")
                for nch in range(E3 // 512):
                    ps = psum1.tile([P, 512], f32, tag="mm1")
                    for k in range(KT):
                        nc.tensor.matmul(
                            ps[:],
                            xT_sb[:, k, tsl],
                            wq_sb[:, k, bass.ts(nch, 512)],
                            start=(k == 0),
                            stop=(k == KT - 1),
                        )
                    nc.scalar.copy(qkv[:, bass.ts(nch, 512)], ps[:])

                qv3 = qkv[:, 0:D].rearrange("p (h d) -> p h d", d=HD)
                kv3 = qkv[:, D : 2 * D].rearrange("p (g d) -> p g d", d=HD)
                # one 2x-mode mul for all (h,g) products, then 2x pairwise tree
                big = scratch.tile([P, H * H * HD], bf, tag="big")
                t1 = scratch.tile([P, 8192], bf, tag="t1")
                t2 = scratch.tile([P, 4096], bf, tag="t2")
                t3 = scratch.tile([P, 2048], bf, tag="t3")
                t4 = scratch.tile([P, 1024], bf, tag="t4")
                t5 = scratch.tile([P, 512], bf, tag="t5")
                bigS = big[:].rearrange("p (h g d) -> p h g d", h=H, g=H)
                nc.vector.tensor_mul(
                    bigS,
                    qv3[:, :, None, :].broadcast_to((P, H, H, HD)),
                    kv3[:, None, :, :].broadcast_to((P, H, H, HD)),
                )
                T1 = t1[:].rearrange("p (h g d) -> p h g d", h=H, g=H)
                T2 = t2[:].rearrange("p (h g d) -> p h g d", h=H, g=H)
                T3 = t3[:].rearrange("p (h g d) -> p h g d", h=H, g=H)
                T4 = t4[:].rearrange("p (h g d) -> p h g d", h=H, g=H)
                T5 = t5[:].rearrange("p (h g d) -> p h g d", h=H, g=H)
                nc.vector.tensor_add(T1, bigS[:, :, :, 0:32], bigS[:, :, :, 32:64])
                nc.vector.tensor_add(T2, T1[:, :, :, 0:16], T1[:, :, :, 16:32])
                nc.vector.tensor_add(T3, T2[:, :, :, 0:8], T2[:, :, :, 8:16])
                nc.vector.tensor_add(T4, T3[:, :, :, 0:4], T3[:, :, :, 4:8])
                nc.vector.tensor_add(T5, T4[:, :, :, 0:2], T4[:, :, :, 2:4])
                scores = pool.tile([P, H, H], f32, tag="sc")  # [p, h, g]
                nc.vector.tensor_add(
                    scores[:, :, :, None], T5[:, :, :, 0:1], T5[:, :, :, 1:2]
                )

                # softmax over g (innermost of [p, h, g]); no max-subtract
                we = pool.tile([P, H, H], bf, tag="we")  # exp scores [p, h, g]
                den = pool.tile([P, H], f32, tag="den")
                rec = pool.tile([P, H], f32, tag="rec")
                wn = pool.tile([P, H, H], bf, tag="wn")  # normalized w [p, h, g]
                nc.scalar.activation(
                    we[:], scores[:], mybir.ActivationFunctionType.Exp, scale=1.0 / 32.0
                )
                nc.vector.reduce_sum(den[:, :, None], we[:], axis=X)
                nc.vector.reciprocal(rec[:], den[:])
                recb = rec[:, :, None].broadcast_to((P, H, H))
                nc.gpsimd.tensor_mul(wn[:], we[:], recb)
                stage2_state[i] = (qkv, wn, big, t1, t2, t3)

            def stage2(i):
                tsl = bass.ts(i, P)
                qkv, wn, big, t1, t2, t3 = stage2_state.pop(i)
                # attn[p, h, d] = sum_g wn[p, h, g] * v[p, d, g]
                # (host pre-permuted the V weight block so v lands as [d, g])
                vv = qkv[:, 2 * D : 3 * D].rearrange("p (d g) -> p d g", g=H)
                bigA = big[:].rearrange("p (h d g) -> p h d g", h=H, d=HD)
                nc.vector.tensor_mul(
                    bigA,
                    wn[:, :, None, :].broadcast_to((P, H, HD, H)),
                    vv[:, None, :, :].broadcast_to((P, H, HD, H)),
                )
                A1 = t1[:].rearrange("p (h d g) -> p h d g", h=H, d=HD)
                A2 = t2[:].rearrange("p (h d g) -> p h d g", h=H, d=HD)
                A3 = t3[:].rearrange("p (h d g) -> p h d g", h=H, d=HD)
                attn_dve = pool.tile([P, H, HD], bf, tag="attn_dve")
                nc.vector.tensor_add(A1, bigA[:, :, :, 0:8], bigA[:, :, :, 8:16])
                nc.vector.tensor_add(A2, A1[:, :, :, 0:4], A1[:, :, :, 4:8])
                nc.vector.tensor_add(A3, A2[:, :, :, 0:2], A2[:, :, :, 2:4])
                nc.vector.tensor_add(
                    attn_dve[:, :, :, None], A3[:, :, :, 0:1], A3[:, :, :, 1:2]
                )

                # transpose attn tile -> [e, t] blocks for output proj
                attnb_flat = attn_dve.rearrange("p h d -> p (h d)")
                attnT = pool.tile([P, KT, P], bf, tag="attnT")
                for c in range(KT):
                    pt = psum2.tile([P, P], bf, tag="pt")
                    nc.tensor.transpose(pt[:], attnb_flat[:, bass.ts(c, P)], ident[:])
                    nc.scalar.copy(attnT[:, c, :], pt[:])

                outt = pool.tile([P, D], f32, tag="outt")
                for nch in range(D // 512):
                    po = psum3.tile([P, 512], f32, tag="po")
                    for k in range(KT):
                        nc.tensor.matmul(
                            po[:],
                            attnT[:, k, :],
                            wo_sb[:, k, bass.ts(nch, 512)],
                            start=(k == 0),
                            stop=(k == KT - 1),
                        )
                    nc.scalar.copy(outt[:, bass.ts(nch, 512)], po[:])
                nc.sync.dma_start(out=out_d[tsl, :], in_=outt[:])

            stage1(0)
            for i in range(1, NT):
                stage1(i)
                stage2(i - 1)
            stage2(NT - 1)
    nc.compile()
    return nc, names


def kernel(x, Wqkv, Wo, bo, trace=False):
    if "nc" not in _CACHE:
        _CACHE["nc"], _CACHE["names"] = _build()
    nc, names = _CACHE["nc"], _CACHE["names"]
    bf = ml_dtypes.bfloat16
    xt = np.ascontiguousarray(
        np.asarray(x, dtype=np.float32).reshape(B * N, D).T
    )  # [D, B*N]
    wqkvT_f = np.ascontiguousarray(np.asarray(Wqkv, dtype=np.float32).T)  # [D, 3D]
    vblk = wqkvT_f[:, 2 * D :].reshape(D, H, HD)
    wqkvT_f[:, 2 * D :] = np.ascontiguousarray(vblk.transpose(0, 2, 1)).reshape(D, D)
    wqkvT = wqkvT_f.astype(bf)
    woT = np.ascontiguousarray(np.asarray(Wo, dtype=np.float32).T).astype(bf)
    in_maps = []
    for c in range(NCORES):
        shard = np.ascontiguousarray(xt[:, c * T : (c + 1) * T]).astype(bf)
        in_maps.append(
            {names["xT"]: shard, names["wqkvT"]: wqkvT, names["woT"]: woT}
        )
    res = run_bass_kernel_spmd(
        nc, in_maps, core_ids=list(range(NCORES)), trace=trace
    )
    shards = [res.results[c][names["out"]] for c in range(NCORES)]
    out = np.concatenate(shards, axis=0).reshape(B, N, D).astype(np.float32)
    out = out + np.asarray(bo, dtype=np.float32)[None, None, :]
    if trace:
        return out, res
    return out


# revision 6
# speedup vs baseline: 1.7151x; 1.0187x over previous
import sys
from contextlib import ExitStack

sys.path.insert(0, "/opt/trn_rl_repo")

import numpy as np
import ml_dtypes

import concourse.bass as bass
import concourse.bacc as bacc
import concourse.mybir as mybir
import concourse.tile as tile
from concourse.bass_utils import run_bass_kernel_spmd
from concourse.masks import make_identity

B, N, D, H = 4, 4096, 1024, 16
HD = D // H
NCORES = 8
T = (B * N) // NCORES  # 2048 tokens per core
P = 128
NT = T // P            # 16 token tiles per core
KT = D // P            # 8 contraction tiles
E3 = 3 * D

GP_ATTN = 0   # gpsimd offload disabled: SBUF-BW contention nets zero

_CACHE = {}


def _name(t):
    return t.name if hasattr(t, "name") else t.tensor.name


def _build():
    bf = mybir.dt.bfloat16
    f32 = mybir.dt.float32
    X = mybir.AxisListType.X
    nc = bacc.Bacc(None, target_bir_lowering=False)
    names = {}
    with tile.TileContext(nc) as tc:
        with ExitStack() as ctx:
            dram = ctx.enter_context(tc.tile_pool(name="dram", bufs=1, space="DRAM"))
            xT_d = dram.tile([D, T], bf, kind="ExternalInput")
            wq_d = dram.tile([D, E3], bf, kind="ExternalInput")
            wo_d = dram.tile([D, D], bf, kind="ExternalInput")
            out_d = dram.tile([T, D], f32, kind="ExternalOutput")
            names["xT"] = _name(xT_d)
            names["wqkvT"] = _name(wq_d)
            names["woT"] = _name(wo_d)
            names["out"] = _name(out_d)

            consts = ctx.enter_context(tc.tile_pool(name="consts", bufs=1))
            xT_sb = consts.tile([P, KT, T], bf)
            wq_sb = consts.tile([P, KT, E3], bf)
            wo_sb = consts.tile([P, KT, D], bf)
            ident = consts.tile([P, P], bf)
            make_identity(nc, ident)
            # split input DMAs so tile-0 compute starts as soon as its
            # slices land (whole-tensor uploads cost ~47us of head idle)
            nc.sync.dma_start(
                out=xT_sb[:, :, 0:P],
                in_=xT_d[:, 0:P].rearrange("(k p) t -> p k t", p=P),
            )
            for nch in range(E3 // 512):
                esl = bass.ts(nch, 512)
                nc.sync.dma_start(
                    out=wq_sb[:, :, esl],
                    in_=wq_d[:, esl].rearrange("(k p) e -> p k e", p=P),
                )
            for i in range(1, NT):
                tsl = bass.ts(i, P)
                nc.sync.dma_start(
                    out=xT_sb[:, :, tsl],
                    in_=xT_d[:, tsl].rearrange("(k p) t -> p k t", p=P),
                )
            nc.sync.dma_start(out=wo_sb[:], in_=wo_d[:].rearrange("(k p) e -> p k e", p=P))

            pool = ctx.enter_context(tc.tile_pool(name="work", bufs=2))
            scratch = ctx.enter_context(tc.tile_pool(name="scratch", bufs=1))
            psum1 = ctx.enter_context(tc.tile_pool(name="psum1", bufs=2, space="PSUM"))
            psum2 = ctx.enter_context(tc.tile_pool(name="psum2", bufs=2, space="PSUM"))
            psum3 = ctx.enter_context(tc.tile_pool(name="psum3", bufs=2, space="PSUM"))

            # stage1(i): qkv proj + score muls + reduces + softmax -> wn
            # stage2(i): attn (DVE+GP) + merge + transpose + out proj
            # Emitted as s1(0), s1(1), s2(0), s1(2), s2(1), ... so DVE always
            # has stage-1 work queued while GPSIMD chews on stage-2 groups.
            stage2_state = {}

            def stage1(i):
                tsl = bass.ts(i, P)
                qkv = pool.tile([P, E3], bf, tag="qkv")
                for nch in range(E3 // 512):
                    ps = psum1.tile([P, 512], f32, tag="mm1")
                    for k in range(KT):
                        nc.tensor.matmul(
                            ps[:],
                            xT_sb[:, k, tsl],
                            wq_sb[:, k, bass.ts(nch, 512)],
                            start=(k == 0),
                            stop=(k == KT - 1),
                        )
                    nc.scalar.copy(qkv[:, bass.ts(nch, 512)], ps[:])

                qv3 = qkv[:, 0:D].rearrange("p (h d) -> p h d", d=HD)
                kv3 = qkv[:, D : 2 * D].rearrange("p (g d) -> p g d", d=HD)
                # one 2x-mode mul for all (h,g) products, then 2x pairwise tree
                big = scratch.tile([P, H * H * HD], bf, tag="big")
                t1 = scratch.tile([P, 8192], bf, tag="t1")
                t2 = scratch.tile([P, 4096], bf, tag="t2")
                t3 = scratch.tile([P, 2048], bf, tag="t3")
                t4 = scratch.tile([P, 1024], bf, tag="t4")
                t5 = scratch.tile([P, 512], bf, tag="t5")
                bigS = big[:].rearrange("p (h g d) -> p h g d", h=H, g=H)
                nc.vector.tensor_mul(
                    bigS,
                    qv3[:, :, None, :].broadcast_to((P, H, H, HD)),
                    kv3[:, None, :, :].broadcast_to((P, H, H, HD)),
                )
                T1 = t1[:].rearrange("p (h g d) -> p h g d", h=H, g=H)
                T2 = t2[:].rearrange("p (h g d) -> p h g d", h=H, g=H)
                T3 = t3[:].rearrange("p (h g d) -> p h g d", h=H, g=H)
                T4 = t4[:].rearrange("p (h g d) -> p h g d", h=H, g=H)
                T5 = t5[:].rearrange("p (h g d) -> p h g d", h=H, g=H)
                nc.vector.tensor_add(T1, bigS[:, :, :, 0:32], bigS[:, :, :, 32:64])
                nc.vector.tensor_add(T2, T1[:, :, :, 0:16], T1[:, :, :, 16:32])
                nc.vector.tensor_add(T3, T2[:, :, :, 0:8], T2[:, :, :, 8:16])
                nc.vector.tensor_add(T4, T3[:, :, :, 0:4], T3[:, :, :, 4:8])
                nc.vector.tensor_add(T5, T4[:, :, :, 0:2], T4[:, :, :, 2:4])
                scores = pool.tile([P, H, H], f32, tag="sc")  # [p, h, g]
                nc.vector.tensor_add(
                    scores[:, :, :, None], T5[:, :, :, 0:1], T5[:, :, :, 1:2]
                )

                # softmax over g (innermost of [p, h, g]); no max-subtract
                we = pool.tile([P, H, H], bf, tag="we")  # exp scores [p, h, g]
                den = pool.tile([P, H], f32, tag="den")
                rec = pool.tile([P, H], f32, tag="rec")
                wn = pool.tile([P, H, H], bf, tag="wn")  # normalized w [p, h, g]
                nc.scalar.activation(
                    we[:], scores[:], mybir.ActivationFunctionType.Exp, scale=1.0 / 32.0
                )
                nc.vector.reduce_sum(den[:, :, None], we[:], axis=X)
                nc.vector.reciprocal(rec[:], den[:])
                recb = rec[:, :, None].broadcast_to((P, H, H))
                nc.gpsimd.tensor_mul(wn[:], we[:], recb)
                stage2_state[i] = (qkv, wn, big, t1, t2, t3)

            def stage2(i):
                tsl = bass.ts(i, P)
                qkv, wn, big, t1, t2, t3 = stage2_state.pop(i)
                # attn[p, h, d] = sum_g wn[p, h, g] * v[p, d, g]
                # (host pre-permuted the V weight block so v lands as [d, g])
                vv = qkv[:, 2 * D : 3 * D].rearrange("p (d g) -> p d g", g=H)
                bigA = big[:].rearrange("p (h d g) -> p h d g", h=H, d=HD)
                nc.vector.tensor_mul(
                    bigA,
                    wn[:, :, None, :].broadcast_to((P, H, HD, H)),
                    vv[:, None, :, :].broadcast_to((P, H, HD, H)),
                )
                A1 = t1[:].rearrange("p (h d g) -> p h d g", h=H, d=HD)
                A2 = t2[:].rearrange("p (h d g) -> p h d g", h=H, d=HD)
                A3 = t3[:].rearrange("p (h d g) -> p h d g", h=H, d=HD)
                attn_dve = pool.tile([P, H, HD], bf, tag="attn_dve")
                nc.vector.tensor_add(A1, bigA[:, :, :, 0:8], bigA[:, :, :, 8:16])
                nc.vector.tensor_add(A2, A1[:, :, :, 0:4], A1[:, :, :, 4:8])
                nc.vector.tensor_add(A3, A2[:, :, :, 0:2], A2[:, :, :, 2:4])
                nc.vector.tensor_add(
                    attn_dve[:, :, :, None], A3[:, :, :, 0:1], A3[:, :, :, 1:2]
                )

                # transpose attn tile -> [e, t] blocks for output proj
                attnb_flat = attn_dve.rearrange("p h d -> p (h d)")
                attnT = pool.tile([P, KT, P], bf, tag="attnT")
                for c in range(KT):
                    pt = psum2.tile([P, P], bf, tag="pt")
                    nc.tensor.transpose(pt[:], attnb_flat[:, bass.ts(c, P)], ident[:])
                    nc.scalar.copy(attnT[:, c, :], pt[:])

                outt = pool.tile([P, D], f32, tag="outt")
                for nch in range(D // 512):
                    po = psum3.tile([P, 512], f32, tag="po")
                    for k in range(KT):
                        nc.tensor.matmul(
                            po[:],
                            attnT[:, k, :],
                            wo_sb[:, k, bass.ts(nch, 512)],
                            start=(k == 0),
                            stop=(k == KT - 1),
                        )
                    nc.scalar.copy(outt[:, bass.ts(nch, 512)], po[:])
                nc.sync.dma_start(out=out_d[tsl, :], in_=outt[:])

            stage1(0)
            for i in range(1, NT):
                stage1(i)
                stage2(i - 1)
            stage2(NT - 1)
    nc.compile()
    return nc, names


def kernel(x, Wqkv, Wo, bo, trace=False):
    if "nc" not in _CACHE:
        _CACHE["nc"], _CACHE["names"] = _build()
    nc, names = _CACHE["nc"], _CACHE["names"]
    bf = ml_dtypes.bfloat16
    xt = np.ascontiguousarray(
        np.asarray(x, dtype=np.float32).reshape(B * N, D).T
    )  # [D, B*N]
    wqkvT_f = np.ascontiguousarray(np.asarray(Wqkv, dtype=np.float32).T)  # [D, 3D]
    vblk = wqkvT_f[:, 2 * D :].reshape(D, H, HD)
    wqkvT_f[:, 2 * D :] = np.ascontiguousarray(vblk.transpose(0, 2, 1)).reshape(D, D)
    wqkvT = wqkvT_f.astype(bf)
    woT = np.ascontiguousarray(np.asarray(Wo, dtype=np.float32).T).astype(bf)
    in_maps = []
    for c in range(NCORES):
        shard = np.ascontiguousarray(xt[:, c * T : (c + 1) * T]).astype(bf)
        in_maps.append(
            {names["xT"]: shard, names["wqkvT"]: wqkvT, names["woT"]: woT}
        )
    res = run_bass_kernel_spmd(
        nc, in_maps, core_ids=list(range(NCORES)), trace=trace
    )
    shards = [res.results[c][names["out"]] for c in range(NCORES)]
    out = np.concatenate(shards, axis=0).reshape(B, N, D).astype(np.float32)
    out = out + np.asarray(bo, dtype=np.float32)[None, None, :]
    if trace:
        return out, res
    return out


# revision 7
# speedup vs baseline: 1.7274x; 1.0072x over previous
import sys
from contextlib import ExitStack

sys.path.insert(0, "/opt/trn_rl_repo")

import numpy as np
import ml_dtypes

import concourse.bass as bass
import concourse.bacc as bacc
import concourse.mybir as mybir
import concourse.tile as tile
from concourse.bass_utils import run_bass_kernel_spmd
from concourse.masks import make_identity

B, N, D, H = 4, 4096, 1024, 16
HD = D // H
NCORES = 8
T = (B * N) // NCORES  # 2048 tokens per core
P = 128
NT = T // P            # 16 token tiles per core
KT = D // P            # 8 contraction tiles
E3 = 3 * D

GP_ATTN = 0   # gpsimd offload disabled: SBUF-BW contention nets zero

_CACHE = {}


def _name(t):
    return t.name if hasattr(t, "name") else t.tensor.name


def _build():
    bf = mybir.dt.bfloat16
    f32 = mybir.dt.float32
    X = mybir.AxisListType.X
    nc = bacc.Bacc(None, target_bir_lowering=False)
    names = {}
    with tile.TileContext(nc) as tc:
        with ExitStack() as ctx:
            dram = ctx.enter_context(tc.tile_pool(name="dram", bufs=1, space="DRAM"))
            xT_d = dram.tile([D, T], bf, kind="ExternalInput")
            wq_d = dram.tile([D, E3], bf, kind="ExternalInput")
            wo_d = dram.tile([D, D], bf, kind="ExternalInput")
            out_d = dram.tile([T, D], f32, kind="ExternalOutput")
            names["xT"] = _name(xT_d)
            names["wqkvT"] = _name(wq_d)
            names["woT"] = _name(wo_d)
            names["out"] = _name(out_d)

            consts = ctx.enter_context(tc.tile_pool(name="consts", bufs=1))
            xT_sb = consts.tile([P, KT, T], bf)
            wq_sb = consts.tile([P, KT, E3], bf)
            wo_sb = consts.tile([P, KT, D], bf)
            ident = consts.tile([P, P], bf)
            make_identity(nc, ident)
            # split input DMAs so tile-0 compute starts as soon as its
            # slices land (whole-tensor uploads cost ~47us of head idle)
            nc.sync.dma_start(
                out=xT_sb[:, :, 0:P],
                in_=xT_d[:, 0:P].rearrange("(k p) t -> p k t", p=P),
            )
            for nch in range(E3 // 512):
                esl = bass.ts(nch, 512)
                nc.sync.dma_start(
                    out=wq_sb[:, :, esl],
                    in_=wq_d[:, esl].rearrange("(k p) e -> p k e", p=P),
                )
            for i in range(1, NT):
                tsl = bass.ts(i, P)
                nc.sync.dma_start(
                    out=xT_sb[:, :, tsl],
                    in_=xT_d[:, tsl].rearrange("(k p) t -> p k t", p=P),
                )
            nc.sync.dma_start(out=wo_sb[:], in_=wo_d[:].rearrange("(k p) e -> p k e", p=P))

            pool = ctx.enter_context(tc.tile_pool(name="work", bufs=2))
            scratch = ctx.enter_context(tc.tile_pool(name="scratch", bufs=1))
            psum1 = ctx.enter_context(tc.tile_pool(name="psum1", bufs=2, space="PSUM"))
            psum2 = ctx.enter_context(tc.tile_pool(name="psum2", bufs=2, space="PSUM"))
            psum3 = ctx.enter_context(tc.tile_pool(name="psum3", bufs=2, space="PSUM"))

            # warm the PE HAM clock-gate while input DMAs stream
            for _ in range(10):
                wp = psum2.tile([P, P], bf, tag="pt")
                nc.tensor.transpose(wp[:], ident[:], ident[:])

            # stage1(i): qkv proj + score muls + reduces + softmax -> wn
            # stage2(i): attn (DVE+GP) + merge + transpose + out proj
            # Emitted as s1(0), s1(1), s2(0), s1(2), s2(1), ... so DVE always
            # has stage-1 work queued while GPSIMD chews on stage-2 groups.
            stage2_state = {}

            def stage1(i):
                tsl = bass.ts(i, P)
                qkv = pool.tile([P, E3], bf, tag="qkv")
                for nch in range(E3 // 512):
                    ps = psum1.tile([P, 512], f32, tag="mm1")
                    for k in range(KT):
                        nc.tensor.matmul(
                            ps[:],
                            xT_sb[:, k, tsl],
                            wq_sb[:, k, bass.ts(nch, 512)],
                            start=(k == 0),
                            stop=(k == KT - 1),
                        )
                    nc.scalar.copy(qkv[:, bass.ts(nch, 512)], ps[:])

                qv3 = qkv[:, 0:D].rearrange("p (h d) -> p h d", d=HD)
                kv3 = qkv[:, D : 2 * D].rearrange("p (g d) -> p g d", d=HD)
                # one 2x-mode mul for all (h,g) products, then 2x pairwise tree
                big = scratch.tile([P, H * H * HD], bf, tag="big")
                t1 = scratch.tile([P, 8192], bf, tag="t1")
                t2 = scratch.tile([P, 4096], bf, tag="t2")
                t3 = scratch.tile([P, 2048], bf, tag="t3")
                t4 = scratch.tile([P, 1024], bf, tag="t4")
                t5 = scratch.tile([P, 512], bf, tag="t5")
                bigS = big[:].rearrange("p (h g d) -> p h g d", h=H, g=H)
                nc.vector.tensor_mul(
                    bigS,
                    qv3[:, :, None, :].broadcast_to((P, H, H, HD)),
                    kv3[:, None, :, :].broadcast_to((P, H, H, HD)),
                )
                T1 = t1[:].rearrange("p (h g d) -> p h g d", h=H, g=H)
                T2 = t2[:].rearrange("p (h g d) -> p h g d", h=H, g=H)
                T3 = t3[:].rearrange("p (h g d) -> p h g d", h=H, g=H)
                T4 = t4[:].rearrange("p (h g d) -> p h g d", h=H, g=H)
                T5 = t5[:].rearrange("p (h g d) -> p h g d", h=H, g=H)
                nc.vector.tensor_add(T1, bigS[:, :, :, 0:32], bigS[:, :, :, 32:64])
                nc.vector.tensor_add(T2, T1[:, :, :, 0:16], T1[:, :, :, 16:32])
                nc.vector.tensor_add(T3, T2[:, :, :, 0:8], T2[:, :, :, 8:16])
                nc.vector.tensor_add(T4, T3[:, :, :, 0:4], T3[:, :, :, 4:8])
                nc.vector.tensor_add(T5, T4[:, :, :, 0:2], T4[:, :, :, 2:4])
                scores = pool.tile([P, H, H], f32, tag="sc")  # [p, h, g]
                nc.vector.tensor_add(
                    scores[:, :, :, None], T5[:, :, :, 0:1], T5[:, :, :, 1:2]
                )

                # softmax over g (innermost of [p, h, g]); no max-subtract
                we = pool.tile([P, H, H], bf, tag="we")  # exp scores [p, h, g]
                den = pool.tile([P, H], f32, tag="den")
                rec = pool.tile([P, H], f32, tag="rec")
                wn = pool.tile([P, H, H], bf, tag="wn")  # normalized w [p, h, g]
                nc.scalar.activation(
                    we[:], scores[:], mybir.ActivationFunctionType.Exp, scale=1.0 / 32.0
                )
                stage2_state[i] = (qkv, we, den, rec, wn, big, t1, t2, t3)

            def stage2(i):
                tsl = bass.ts(i, P)
                qkv, we, den, rec, wn, big, t1, t2, t3 = stage2_state.pop(i)
                nc.vector.reduce_sum(den[:, :, None], we[:], axis=X)
                nc.vector.reciprocal(rec[:], den[:])
                recb = rec[:, :, None].broadcast_to((P, H, H))
                nc.vector.tensor_mul(wn[:], we[:], recb)
                # attn[p, h, d] = sum_g wn[p, h, g] * v[p, d, g]
                # (host pre-permuted the V weight block so v lands as [d, g])
                vv = qkv[:, 2 * D : 3 * D].rearrange("p (d g) -> p d g", g=H)
                bigA = big[:].rearrange("p (h d g) -> p h d g", h=H, d=HD)
                nc.vector.tensor_mul(
                    bigA,
                    wn[:, :, None, :].broadcast_to((P, H, HD, H)),
                    vv[:, None, :, :].broadcast_to((P, H, HD, H)),
                )
                A1 = t1[:].rearrange("p (h d g) -> p h d g", h=H, d=HD)
                A2 = t2[:].rearrange("p (h d g) -> p h d g", h=H, d=HD)
                A3 = t3[:].rearrange("p (h d g) -> p h d g", h=H, d=HD)
                attn_dve = pool.tile([P, H, HD], bf, tag="attn_dve")
                nc.vector.tensor_add(A1, bigA[:, :, :, 0:8], bigA[:, :, :, 8:16])
                nc.vector.tensor_add(A2, A1[:, :, :, 0:4], A1[:, :, :, 4:8])
                nc.vector.tensor_add(A3, A2[:, :, :, 0:2], A2[:, :, :, 2:4])
                nc.vector.tensor_add(
                    attn_dve[:, :, :, None], A3[:, :, :, 0:1], A3[:, :, :, 1:2]
                )

                # transpose attn tile -> [e, t] blocks for output proj
                attnb_flat = attn_dve.rearrange("p h d -> p (h d)")
                attnT = pool.tile([P, KT, P], bf, tag="attnT")
                for c in range(KT):
                    pt = psum2.tile([P, P], bf, tag="pt")
                    nc.tensor.transpose(pt[:], attnb_flat[:, bass.ts(c, P)], ident[:])
                    nc.scalar.copy(attnT[:, c, :], pt[:])

                outt = pool.tile([P, D], f32, tag="outt")
                for nch in range(D // 512):
                    po = psum3.tile([P, 512], f32, tag="po")
                    for k in range(KT):
                        nc.tensor.matmul(
                            po[:],
                            attnT[:, k, :],
                            wo_sb[:, k, bass.ts(nch, 512)],
                            start=(k == 0),
                            stop=(k == KT - 1),
                        )
                    nc.scalar.copy(outt[:, bass.ts(nch, 512)], po[:])
                nc.sync.dma_start(out=out_d[tsl, :], in_=outt[:])

            stage1(0)
            for i in range(1, NT):
                stage1(i)
                stage2(i - 1)
            stage2(NT - 1)
    nc.compile()
    return nc, names


def kernel(x, Wqkv, Wo, bo, trace=False):
    if "nc" not in _CACHE:
        _CACHE["nc"], _CACHE["names"] = _build()
    nc, names = _CACHE["nc"], _CACHE["names"]
    bf = ml_dtypes.bfloat16
    xt = np.ascontiguousarray(
        np.asarray(x, dtype=np.float32).reshape(B * N, D).T
    )  # [D, B*N]
    wqkvT_f = np.ascontiguousarray(np.asarray(Wqkv, dtype=np.float32).T)  # [D, 3D]
    vblk = wqkvT_f[:, 2 * D :].reshape(D, H, HD)
    wqkvT_f[:, 2 * D :] = np.ascontiguousarray(vblk.transpose(0, 2, 1)).reshape(D, D)
    wqkvT = wqkvT_f.astype(bf)
    woT = np.ascontiguousarray(np.asarray(Wo, dtype=np.float32).T).astype(bf)
    in_maps = []
    for c in range(NCORES):
        shard = np.ascontiguousarray(xt[:, c * T : (c + 1) * T]).astype(bf)
        in_maps.append(
            {names["xT"]: shard, names["wqkvT"]: wqkvT, names["woT"]: woT}
        )
    res = run_bass_kernel_spmd(
        nc, in_maps, core_ids=list(range(NCORES)), trace=trace
    )
    shards = [res.results[c][names["out"]] for c in range(NCORES)]
    out = np.concatenate(shards, axis=0).reshape(B, N, D).astype(np.float32)
    out = out + np.asarray(bo, dtype=np.float32)[None, None, :]
    if trace:
        return out, res
    return out


# revision 8
# speedup vs baseline: 1.7311x; 1.0021x over previous
import sys
from contextlib import ExitStack

sys.path.insert(0, "/opt/trn_rl_repo")

import numpy as np
import ml_dtypes

import concourse.bass as bass
import concourse.bacc as bacc
import concourse.mybir as mybir
import concourse.tile as tile
from concourse.bass_utils import run_bass_kernel_spmd
from concourse.masks import make_identity

B, N, D, H = 4, 4096, 1024, 16
HD = D // H
NCORES = 8
T = (B * N) // NCORES  # 2048 tokens per core
P = 128
NT = T // P            # 16 token tiles per core
KT = D // P            # 8 contraction tiles
E3 = 3 * D

GP_ATTN = 0   # gpsimd offload disabled: SBUF-BW contention nets zero

_CACHE = {}


def _name(t):
    return t.name if hasattr(t, "name") else t.tensor.name


def _build():
    bf = mybir.dt.bfloat16
    f32 = mybir.dt.float32
    X = mybir.AxisListType.X
    nc = bacc.Bacc(None, target_bir_lowering=False)
    names = {}
    with tile.TileContext(nc) as tc:
        with ExitStack() as ctx:
            dram = ctx.enter_context(tc.tile_pool(name="dram", bufs=1, space="DRAM"))
            xT_d = dram.tile([D, T], bf, kind="ExternalInput")
            wq_d = dram.tile([D, E3], bf, kind="ExternalInput")
            wo_d = dram.tile([D, D], bf, kind="ExternalInput")
            out_d = dram.tile([T, D], f32, kind="ExternalOutput")
            names["xT"] = _name(xT_d)
            names["wqkvT"] = _name(wq_d)
            names["woT"] = _name(wo_d)
            names["out"] = _name(out_d)

            consts = ctx.enter_context(tc.tile_pool(name="consts", bufs=1))
            xT_sb = consts.tile([P, KT, T], bf)
            wq_sb = consts.tile([P, KT, E3], bf)
            wo_sb = consts.tile([P, KT, D], bf)
            ident = consts.tile([P, P], bf)
            make_identity(nc, ident)
            # split input DMAs so tile-0 compute starts as soon as its
            # slices land (whole-tensor uploads cost ~47us of head idle)
            nc.sync.dma_start(
                out=xT_sb[:, :, 0:P],
                in_=xT_d[:, 0:P].rearrange("(k p) t -> p k t", p=P),
            )
            for nch in range(E3 // 512):
                esl = bass.ts(nch, 512)
                nc.sync.dma_start(
                    out=wq_sb[:, :, esl],
                    in_=wq_d[:, esl].rearrange("(k p) e -> p k e", p=P),
                )
            for i in range(1, NT):
                tsl = bass.ts(i, P)
                nc.sync.dma_start(
                    out=xT_sb[:, :, tsl],
                    in_=xT_d[:, tsl].rearrange("(k p) t -> p k t", p=P),
                )
            nc.sync.dma_start(out=wo_sb[:], in_=wo_d[:].rearrange("(k p) e -> p k e", p=P))

            pool = ctx.enter_context(tc.tile_pool(name="work", bufs=2))
            qpool = ctx.enter_context(tc.tile_pool(name="qkvp", bufs=3))
            scratch = ctx.enter_context(tc.tile_pool(name="scratch", bufs=1))
            psum1 = ctx.enter_context(tc.tile_pool(name="psum1", bufs=2, space="PSUM"))
            psum2 = ctx.enter_context(tc.tile_pool(name="psum2", bufs=2, space="PSUM"))
            psum3 = ctx.enter_context(tc.tile_pool(name="psum3", bufs=2, space="PSUM"))

            # warm the PE HAM clock-gate while input DMAs stream
            for _ in range(10):
                wp = psum2.tile([P, P], bf, tag="pt")
                nc.tensor.transpose(wp[:], ident[:], ident[:])

            # stage1(i): qkv proj + score muls + reduces + softmax -> wn
            # stage2(i): attn (DVE+GP) + merge + transpose + out proj
            # Emitted as s1(0), s1(1), s2(0), s1(2), s2(1), ... so DVE always
            # has stage-1 work queued while GPSIMD chews on stage-2 groups.
            stage2_state = {}

            qkv_state = {}

            def stage1a(i):
                tsl = bass.ts(i, P)
                qkv = qpool.tile([P, E3], bf, tag="qkv")
                for nch in range(E3 // 512):
                    ps = psum1.tile([P, 512], f32, tag="mm1")
                    for k in range(KT):
                        nc.tensor.matmul(
                            ps[:],
                            xT_sb[:, k, tsl],
                            wq_sb[:, k, bass.ts(nch, 512)],
                            start=(k == 0),
                            stop=(k == KT - 1),
                        )
                    nc.scalar.copy(qkv[:, bass.ts(nch, 512)], ps[:])
                qkv_state[i] = qkv

            def stage1b(i):
                qkv = qkv_state[i]
                qv3 = qkv[:, 0:D].rearrange("p (h d) -> p h d", d=HD)
                kv3 = qkv[:, D : 2 * D].rearrange("p (g d) -> p g d", d=HD)
                # one 2x-mode mul for all (h,g) products, then 2x pairwise tree
                big = scratch.tile([P, H * H * HD], bf, tag="big")
                t1 = scratch.tile([P, 8192], bf, tag="t1")
                t2 = scratch.tile([P, 4096], bf, tag="t2")
                t3 = scratch.tile([P, 2048], bf, tag="t3")
                t4 = scratch.tile([P, 1024], bf, tag="t4")
                t5 = scratch.tile([P, 512], bf, tag="t5")
                bigS = big[:].rearrange("p (h g d) -> p h g d", h=H, g=H)
                nc.vector.tensor_mul(
                    bigS,
                    qv3[:, :, None, :].broadcast_to((P, H, H, HD)),
                    kv3[:, None, :, :].broadcast_to((P, H, H, HD)),
                )
                T1 = t1[:].rearrange("p (h g d) -> p h g d", h=H, g=H)
                T2 = t2[:].rearrange("p (h g d) -> p h g d", h=H, g=H)
                T3 = t3[:].rearrange("p (h g d) -> p h g d", h=H, g=H)
                T4 = t4[:].rearrange("p (h g d) -> p h g d", h=H, g=H)
                T5 = t5[:].rearrange("p (h g d) -> p h g d", h=H, g=H)
                nc.vector.tensor_add(T1, bigS[:, :, :, 0:32], bigS[:, :, :, 32:64])
                nc.vector.tensor_add(T2, T1[:, :, :, 0:16], T1[:, :, :, 16:32])
                nc.vector.tensor_add(T3, T2[:, :, :, 0:8], T2[:, :, :, 8:16])
                nc.vector.tensor_add(T4, T3[:, :, :, 0:4], T3[:, :, :, 4:8])
                nc.vector.tensor_add(T5, T4[:, :, :, 0:2], T4[:, :, :, 2:4])
                scores = pool.tile([P, H, H], f32, tag="sc")  # [p, h, g]
                nc.vector.tensor_add(
                    scores[:, :, :, None], T5[:, :, :, 0:1], T5[:, :, :, 1:2]
                )

                # softmax over g (innermost of [p, h, g]); no max-subtract
                we = pool.tile([P, H, H], bf, tag="we")  # exp scores [p, h, g]
                den = pool.tile([P, H], f32, tag="den")
                rec = pool.tile([P, H], f32, tag="rec")
                wn = pool.tile([P, H, H], bf, tag="wn")  # normalized w [p, h, g]
                nc.scalar.activation(
                    we[:], scores[:], mybir.ActivationFunctionType.Exp, scale=1.0 / 32.0
                )
                stage2_state[i] = (we, den, rec, wn, big, t1, t2, t3)

            def stage2(i):
                tsl = bass.ts(i, P)
                we, den, rec, wn, big, t1, t2, t3 = stage2_state.pop(i)
                qkv = qkv_state.pop(i)
                nc.vector.reduce_sum(den[:, :, None], we[:], axis=X)
                nc.vector.reciprocal(rec[:], den[:])
                recb = rec[:, :, None].broadcast_to((P, H, H))
                nc.vector.tensor_mul(wn[:], we[:], recb)
                # attn[p, h, d] = sum_g wn[p, h, g] * v[p, d, g]
                # (host pre-permuted the V weight block so v lands as [d, g])
                vv = qkv[:, 2 * D : 3 * D].rearrange("p (d g) -> p d g", g=H)
                bigA = big[:].rearrange("p (h d g) -> p h d g", h=H, d=HD)
                nc.vector.tensor_mul(
                    bigA,
                    wn[:, :, None, :].broadcast_to((P, H, HD, H)),
                    vv[:, None, :, :].broadcast_to((P, H, HD, H)),
                )
                A1 = t1[:].rearrange("p (h d g) -> p h d g", h=H, d=HD)
                A2 = t2[:].rearrange("p (h d g) -> p h d g", h=H, d=HD)
                A3 = t3[:].rearrange("p (h d g) -> p h d g", h=H, d=HD)
                attn_dve = pool.tile([P, H, HD], bf, tag="attn_dve")
                nc.vector.tensor_add(A1, bigA[:, :, :, 0:8], bigA[:, :, :, 8:16])
                nc.vector.tensor_add(A2, A1[:, :, :, 0:4], A1[:, :, :, 4:8])
                nc.vector.tensor_add(A3, A2[:, :, :, 0:2], A2[:, :, :, 2:4])
                nc.vector.tensor_add(
                    attn_dve[:, :, :, None], A3[:, :, :, 0:1], A3[:, :, :, 1:2]
                )

                # transpose attn tile -> [e, t] blocks for output proj
                attnb_flat = attn_dve.rearrange("p h d -> p (h d)")
                attnT = pool.tile([P, KT, P], bf, tag="attnT")
                for c in range(KT):
                    pt = psum2.tile([P, P], bf, tag="pt")
                    nc.tensor.transpose(pt[:], attnb_flat[:, bass.ts(c, P)], ident[:])
                    nc.scalar.copy(attnT[:, c, :], pt[:])

                outt = pool.tile([P, D], f32, tag="outt")
                for nch in range(D // 512):
                    po = psum3.tile([P, 512], f32, tag="po")
                    for k in range(KT):
                        nc.tensor.matmul(
                            po[:],
                            attnT[:, k, :],
                            wo_sb[:, k, bass.ts(nch, 512)],
                            start=(k == 0),
                            stop=(k == KT - 1),
                        )
                    nc.scalar.copy(outt[:, bass.ts(nch, 512)], po[:])
                nc.sync.dma_start(out=out_d[tsl, :], in_=outt[:])

            stage1a(0)
            stage1a(1)
            stage1b(0)
            for i in range(0, NT - 2):
                stage1a(i + 2)
                stage1b(i + 1)
                stage2(i)
            stage1b(NT - 1)
            stage2(NT - 2)
            stage2(NT - 1)
    nc.compile()
    return nc, names


def kernel(x, Wqkv, Wo, bo, trace=False):
    if "nc" not in _CACHE:
        _CACHE["nc"], _CACHE["names"] = _build()
    nc, names = _CACHE["nc"], _CACHE["names"]
    bf = ml_dtypes.bfloat16
    xt = np.ascontiguousarray(
        np.asarray(x, dtype=np.float32).reshape(B * N, D).T
    )  # [D, B*N]
    wqkvT_f = np.ascontiguousarray(np.asarray(Wqkv, dtype=np.float32).T)  # [D, 3D]
    vblk = wqkvT_f[:, 2 * D :].reshape(D, H, HD)
    wqkvT_f[:, 2 * D :] = np.ascontiguousarray(vblk.transpose(0, 2, 1)).reshape(D, D)
    wqkvT = wqkvT_f.astype(bf)
    woT = np.ascontiguousarray(np.asarray(Wo, dtype=np.float32).T).astype(bf)
    in_maps = []
    for c in range(NCORES):
        shard = np.ascontiguousarray(xt[:, c * T : (c + 1) * T]).astype(bf)
        in_maps.append(
            {names["xT"]: shard, names["wqkvT"]: wqkvT, names["woT"]: woT}
        )
    res = run_bass_kernel_spmd(
        nc, in_maps, core_ids=list(range(NCORES)), trace=trace
    )
    shards = [res.results[c][names["out"]] for c in range(NCORES)]
    out = np.concatenate(shards, axis=0).reshape(B, N, D).astype(np.float32)
    out = out + np.asarray(bo, dtype=np.float32)[None, None, :]
    if trace:
        return out, res
    return out
